# Initial kernel scaffold
#
"""Trainium2 Bass kernel for CausalSelfAttention (GQA + alibi, B=2, T=2048,
d_model=2048, 16 q heads / 4 kv heads).

Sharding: 8 cores = (batch b in {0,1}) x (kv-group g in {0..3}).
Each core computes, for its (b, g):
  - QKV^T slice:  [768, T]  (4 q heads pre-scaled by 1/sqrt(hd), 1 k head, 1 v head)
  - causal attention for its 4 query heads (scores kept transposed:
    S^T[j, i] with keys j on partitions, softmax denominator accumulated on
    DVE + log-tree partition reduce, normalization broadcast via a K=1 matmul)
  - partial output projection: O_slice[t, 512] @ proj_w[:, slice]^T -> [T, 2048]
Host sums the 4 partials per batch and adds proj_b.

All matmuls run as float32r (reduced-precision fp32, 1.5 cyc/row on PE).
"""

import math

import numpy as np

D = 2048
T = 2048
NH = 16
KVH = 4
HD = 128
GRP = 4
B = 2
NCORE = 8
FB = 6          # qkv feature tiles of 128 (4 q heads + k + v)
NEG = -1.0e30

_CACHE: dict = {}


# --------------------------------------------------------------------------
# device kernel
# --------------------------------------------------------------------------

def _build_nc():
    import concourse.mybir as mybir
    from concourse import bacc
    import concourse.tile as tile
    from concourse.masks import make_identity

    f32 = mybir.dt.float32
    f32r = mybir.dt.float32r
    Exp = mybir.ActivationFunctionType.Exp
    add = mybir.AluOpType.add
    mult = mybir.AluOpType.mult

    def R(ap):
        return ap.bitcast(f32r)

    nc = bacc.Bacc("TRN2", target_bir_lowering=False, debug=False,
                   num_devices=NCORE)

    xt_d = nc.dram_tensor("xt", [128, 16 * T], f32, kind="ExternalInput").ap()
    wt_d = nc.dram_tensor("wt", [128, 16 * 768], f32, kind="ExternalInput").ap()
    bq_d = nc.dram_tensor("bq", [128, FB], f32, kind="ExternalInput").ap()
    at_d = nc.dram_tensor("at", [128, 20 * 512], f32, kind="ExternalInput").ap()
    cb_d = nc.dram_tensor("cb", [128, 48], f32, kind="ExternalInput").ap()
    pt_d = nc.dram_tensor("pt", [128, 4 * T], f32, kind="ExternalInput").ap()
    out_d = nc.dram_tensor("out", [T, D], f32, kind="ExternalOutput").ap()

    with tile.TileContext(nc) as tc:
        with tc.tile_pool(name="persist", bufs=1) as pp, \
             tc.tile_pool(name="ps", bufs=3, space="PSUM") as ps_pool, \
             tc.tile_pool(name="po", bufs=2, space="PSUM") as po_pool, \
             tc.tile_pool(name="pv", bufs=2, space="PSUM") as pv_pool:

            qkvT = pp.tile([128, FB * T], f32, name="qkvT", tag="qkvT")
            bq = pp.tile([128, FB], f32, name="bqs", tag="bqs")
            cb = pp.tile([128, 48], f32, name="cbs", tag="cbs")
            ident = pp.tile([128, 128], f32, name="ident", tag="ident")
            ones = pp.tile([1, 128], f32, name="ones1", tag="ones1")
            nc.sync.dma_start(bq, bq_d)
            nc.sync.dma_start(cb, cb_d)
            make_identity(nc, ident)
            nc.vector.memset(ones, 1.0)

            # ---------------- stage 1: qkvT = W_c @ x^T + b ----------------
            with tc.tile_pool(name="s1w", bufs=1) as s1w, \
                 tc.tile_pool(name="s1x", bufs=2) as s1x:
                wt = s1w.tile([128, 16 * 768], f32, name="wt", tag="wt")
                for dt_ in range(16):
                    nc.sync.dma_start(wt[:, dt_ * 768:(dt_ + 1) * 768],
                                      wt_d[:, dt_ * 768:(dt_ + 1) * 768])
                for tb in range(4):           # 512-wide t blocks
                    xt = s1x.tile([128, 16 * 512], f32, name="xt", tag="xt")
                    for dt_ in range(16):
                        nc.sync.dma_start(
                            xt[:, dt_ * 512:(dt_ + 1) * 512],
                            xt_d[:, dt_ * T + tb * 512: dt_ * T + tb * 512 + 512])
                    for fb in range(FB):
                        acc = ps_pool.tile([128, 512], f32, name="acc", tag="ps")
                        for dt_ in range(16):
                            nc.tensor.matmul(
                                acc,
                                R(wt[:, dt_ * 768 + fb * 128:
                                        dt_ * 768 + fb * 128 + 128]),
                                R(xt[:, dt_ * 512:(dt_ + 1) * 512]),
                                start=(dt_ == 0), stop=(dt_ == 15))
                        nc.vector.tensor_scalar_add(
                            qkvT[:, fb * T + tb * 512: fb * T + tb * 512 + 512],
                            acc, bq[:, fb:fb + 1])

            # ---------------- stage 2: attention ----------------
            with tc.tile_pool(name="oTp", bufs=1) as oTp:
                oT = oTp.tile([128, 4 * T], f32, name="oT", tag="oT")
                with tc.tile_pool(name="s2a", bufs=1) as s2a, \
                     tc.tile_pool(name="s2w", bufs=2) as s2w:
                    at = s2a.tile([128, 20 * 512], f32, name="at", tag="at")
                    for k in range(20):
                        nc.sync.dma_start(at[:, k * 512:(k + 1) * 512],
                                          at_d[:, k * 512:(k + 1) * 512])
                    for h in range(4):
                        qT = qkvT[:, h * T:(h + 1) * T]
                        kT = qkvT[:, 4 * T:5 * T]
                        vT = qkvT[:, 5 * T:6 * T]
                        v = s2w.tile([128, T], f32, name="v", tag="v")
                        for jt in range(16):
                            pv = pv_pool.tile([128, 128], f32, name="pv", tag="pv")
                            nc.tensor.transpose(
                                pv, vT[:, jt * 128:(jt + 1) * 128], ident)
                            nc.vector.tensor_copy(
                                v[:, jt * 128:(jt + 1) * 128], pv)
                        for ib in range(4):
                            njb = 4 * (ib + 1)
                            opsum = po_pool.tile([128, 512], f32, name="opsum",
                                                 tag="po")
                            dsum = s2w.tile([128, 512], f32, name="dsum",
                                            tag="dsum")
                            for jb in range(njb):
                                spsum = ps_pool.tile([128, 512], f32,
                                                     name="spsum", tag="ps")
                                nc.tensor.matmul(
                                    spsum,
                                    R(kT[:, jb * 128:(jb + 1) * 128]),
                                    R(qT[:, ib * 512:(ib + 1) * 512]),
                                    start=True, stop=True)
                                ssb = s2w.tile([128, 512], f32, name="ssb",
                                               tag="ssb", bufs=3)
                                dd = jb - 4 * ib
                                if dd >= 0:   # diagonal band (masked tiles)
                                    nc.vector.tensor_tensor(
                                        ssb, spsum,
                                        at[:, (4 + h * 4 + dd) * 512:
                                              (5 + h * 4 + dd) * 512], add)
                                    bias = 0.0
                                else:         # strictly-lower blocks
                                    nc.vector.tensor_tensor(
                                        ssb, spsum,
                                        at[:, h * 512:(h + 1) * 512], add)
                                    k_ = 4 * ib - jb
                                    bias = cb[:, h * 12 + k_ - 1:
                                                 h * 12 + k_]
                                psb = s2w.tile([128, 512], f32, name="psb",
                                               tag="psb", bufs=3)
                                nc.scalar.activation(psb, ssb, Exp, bias=bias,
                                                     scale=1.0)
                                if jb == 0:
                                    nc.vector.tensor_copy(dsum, psb)
                                else:
                                    nc.vector.tensor_tensor(dsum, dsum, psb, add)
                                nc.tensor.matmul(
                                    opsum, R(v[:, jb * 128:(jb + 1) * 128]),
                                    R(psb),
                                    start=(jb == 0), stop=(jb == njb - 1),
                                    skip_group_check=True)
                            # softmax denominator: partition log-tree reduce
                            for half in (64, 32, 16, 8, 4, 2, 1):
                                nc.vector.tensor_tensor(
                                    dsum[0:half, :], dsum[0:half, :],
                                    dsum[half:2 * half, :], add)
                            rsum = s2w.tile([1, 512], f32, name="rsum",
                                            tag="rsum")
                            nc.vector.reciprocal(rsum, dsum[0:1, :])
                            rps = ps_pool.tile([128, 512], f32, name="rps",
                                               tag="ps")
                            nc.tensor.matmul(rps, R(ones), R(rsum),
                                             start=True, stop=True)
                            rsb = s2w.tile([128, 512], f32, name="rsb",
                                           tag="rsb")
                            nc.scalar.copy(rsb, rps)
                            nc.vector.tensor_tensor(
                                oT[:, h * T + ib * 512: h * T + ib * 512 + 512],
                                opsum, rsb, mult)

                # ---------------- stage 3: partial proj ----------------
                with tc.tile_pool(name="s3w", bufs=1) as s3w, \
                     tc.tile_pool(name="s3o", bufs=4) as s3o:
                    pt = s3w.tile([128, 4 * T], f32, name="pt", tag="pt")
                    for k in range(8):
                        nc.sync.dma_start(pt[:, k * 1024:(k + 1) * 1024],
                                          pt_d[:, k * 1024:(k + 1) * 1024])
                    for tb in range(16):
                        for ob in range(4):
                            acc2 = ps_pool.tile([128, 512], f32, name="acc2",
                                                tag="ps")
                            for dt_ in range(4):
                                nc.tensor.matmul(
                                    acc2,
                                    R(oT[:, dt_ * T + tb * 128:
                                            dt_ * T + tb * 128 + 128]),
                                    R(pt[:, dt_ * T + ob * 512:
                                            dt_ * T + ob * 512 + 512]),
                                    start=(dt_ == 0), stop=(dt_ == 3))
                            osb = s3o.tile([128, 512], f32, name="osb",
                                           tag="osb")
                            nc.vector.tensor_copy(osb, acc2)
                            nc.sync.dma_start(
                                out_d[tb * 128:(tb + 1) * 128,
                                      ob * 512:(ob + 1) * 512], osb)

    nc.compile()
    return nc


def get_nc():
    if "nc" not in _CACHE:
        _CACHE["nc"] = _build_nc()
    return _CACHE["nc"]


# --------------------------------------------------------------------------
# host-side packing
# --------------------------------------------------------------------------

def _expected_slopes():
    return 2.0 ** (-8.0 * (np.arange(1, NH + 1) / NH))  # float64


def _check_structure(attn_mask, alibi_bias):
    """Return exact float64 alibi slopes if inputs match the expected
    causal-mask + rank-1 alibi structure, else None."""
    am = np.asarray(attn_mask)
    if am.shape != (1, 1, T, T):
        return None
    if not np.array_equal(am[0, 0], np.tril(np.ones((T, T), dtype=bool))):
        return None
    al = np.asarray(alibi_bias, dtype=np.float32)
    if al.shape != (1, NH, T, T):
        return None
    slopes = _expected_slopes()
    if not np.allclose(al[0, :, 0, 1], slopes.astype(np.float32),
                       rtol=1e-6, atol=1e-8):
        return None
    idx = np.arange(T, dtype=np.float64)
    rel = idx[None, :] - idx[:, None]
    for h in range(NH):
        ref = (slopes[h] * rel).astype(np.float32)
        if not np.array_equal(al[0, h], ref):
            if not np.allclose(al[0, h], ref, rtol=1e-5, atol=1e-4):
                return None
    return slopes


def _pack_core_inputs(x, qkv_w, qkv_b, proj_w, slopes):
    x = np.asarray(x, dtype=np.float32)
    qkv_w = np.asarray(qkv_w, dtype=np.float32)
    qkv_b = np.asarray(qkv_b, dtype=np.float32)
    proj_w = np.asarray(proj_w, dtype=np.float32)
    inv = np.float32(1.0 / math.sqrt(HD))

    xts = []
    for b in range(B):
        xt = np.ascontiguousarray(
            x[b].T.reshape(16, 128, T).transpose(1, 0, 2).reshape(128, 16 * T))
        xts.append(xt)

    per_g = []
    jj = np.arange(128, dtype=np.float64)[:, None]
    ii = np.arange(512, dtype=np.float64)[None, :]
    for g in range(KVH):
        Wq = qkv_w[512 * g:512 * (g + 1)] * inv
        Wk = qkv_w[D + 128 * g: D + 128 * (g + 1)]
        Wv = qkv_w[D + 512 + 128 * g: D + 512 + 128 * (g + 1)]
        Wc = np.concatenate([Wq, Wk, Wv], axis=0)          # [768, 2048]
        wt = np.ascontiguousarray(
            Wc.T.reshape(16, 128, 768).transpose(1, 0, 2).reshape(128, 16 * 768))
        bc = np.concatenate([qkv_b[512 * g:512 * (g + 1)] * inv,
                             qkv_b[D + 128 * g: D + 128 * (g + 1)],
                             qkv_b[D + 512 + 128 * g: D + 512 + 128 * (g + 1)]])
        bqp = np.ascontiguousarray(bc.reshape(FB, 128).T)  # [128, 6]

        at = np.empty((128, 20 * 512), dtype=np.float32)
        cbp = np.empty((128, 48), dtype=np.float32)
        for h in range(GRP):
            s = slopes[4 * g + h]
            at[:, h * 512:(h + 1) * 512] = (s * (jj - ii)).astype(np.float32)
            for dd in range(4):
                A = (s * (jj - ii + 128 * dd)).astype(np.float32)
                A[(jj + 128 * dd - ii) > 0] = np.float32(NEG)
                at[:, (4 + h * 4 + dd) * 512:(5 + h * 4 + dd) * 512] = A
            for k_ in range(1, 13):
                cbp[:, h * 12 + k_ - 1] = np.float32(s * (-128.0 * k_))

        ptp = np.ascontiguousarray(
            proj_w[:, 512 * g:512 * (g + 1)].T
            .reshape(4, 128, T).transpose(1, 0, 2).reshape(128, 4 * T))
        per_g.append({"wt": wt, "bq": bqp, "at": at, "cb": cbp, "pt": ptp})

    in_maps = []
    for c in range(NCORE):
        b, g = divmod(c, KVH)
        m = dict(per_g[g])
        m["xt"] = xts[b]
        in_maps.append(m)
    return in_maps


# --------------------------------------------------------------------------
# numpy fallback (only used if inputs don't match the expected structure)
# --------------------------------------------------------------------------

def _numpy_reference(x, attn_mask, alibi_bias, qkv_w, qkv_b, proj_w, proj_b):
    x = np.asarray(x, dtype=np.float32)
    b, t, c = x.shape
    qkv = x @ qkv_w.T + qkv_b
    q = qkv[..., :D].reshape(b, t, KVH, GRP, HD).transpose(0, 2, 3, 1, 4)
    k = qkv[..., D:D + 512].reshape(b, t, KVH, HD).transpose(0, 2, 1, 3)
    v = qkv[..., D + 512:].reshape(b, t, KVH, HD).transpose(0, 2, 1, 3)
    scale = 1.0 / math.sqrt(HD)
    att = np.einsum("bkgtd,bksd->bkgts", q, k).astype(np.float32) * scale
    att = att + np.asarray(alibi_bias).reshape(1, KVH, GRP, t, t)
    mask = np.asarray(attn_mask)[:, :, None]
    att = np.where(mask, att, -np.inf)
    att = att - att.max(axis=-1, keepdims=True)
    np.exp(att, out=att)
    att /= att.sum(axis=-1, keepdims=True)
    out = np.einsum("bkgts,bksd->bkgtd", att, v)
    out = out.transpose(0, 3, 1, 2, 4).reshape(b, t, c)
    return (out @ proj_w.T + proj_b).astype(np.float32)


# --------------------------------------------------------------------------
# entry point
# --------------------------------------------------------------------------

def kernel(x, attn_mask, alibi_bias, qkv_w, qkv_b, proj_w, proj_b):
    from concourse import bass_utils

    slopes = _check_structure(attn_mask, alibi_bias)
    if slopes is None:
        return _numpy_reference(x, attn_mask, alibi_bias, qkv_w, qkv_b,
                                proj_w, proj_b)

    nc = get_nc()
    in_maps = _pack_core_inputs(x, qkv_w, qkv_b, proj_w, slopes)
    res = bass_utils.run_bass_kernel_spmd(nc, in_maps,
                                          core_ids=list(range(NCORE)))
    proj_b = np.asarray(proj_b, dtype=np.float32)
    out = np.empty((B, T, D), dtype=np.float32)
    for b in range(B):
        acc = res.results[4 * b + 0]["out"].astype(np.float32, copy=True)
        for g in range(1, KVH):
            acc += res.results[4 * b + g]["out"]
        out[b] = acc + proj_b
    return out


# revision 5
# speedup vs baseline: 1.0045x; 1.0045x over previous
"""Trainium2 Bass kernel for CausalSelfAttention (GQA + alibi, B=2, T=2048,
d_model=2048, 16 q heads / 4 kv heads).

Sharding: 8 cores = (batch b in {0,1}) x (kv-group g in {0..3}).
Each core computes, for its (b, g):
  - QKV^T slice:  [768, T]  (4 q heads pre-scaled by 1/sqrt(hd), 1 k head, 1 v head)
  - causal attention for its 4 query heads (scores kept transposed:
    S^T[j, i] with keys j on partitions, softmax denominator accumulated on
    DVE + log-tree partition reduce, normalization broadcast via a K=1 matmul)
  - partial output projection: O_slice[t, 512] @ proj_w[:, slice]^T -> [T, 2048]
Host sums the 4 partials per batch and adds proj_b.

All matmuls run as float32r (reduced-precision fp32, 1.5 cyc/row on PE).
"""

import math

import numpy as np

D = 2048
T = 2048
NH = 16
KVH = 4
HD = 128
GRP = 4
B = 2
NCORE = 8
FB = 6          # qkv feature tiles of 128 (4 q heads + k + v)
NEG = -1.0e30

_CACHE: dict = {}


# --------------------------------------------------------------------------
# device kernel
# --------------------------------------------------------------------------

def _build_nc():
    import concourse.mybir as mybir
    from concourse import bacc
    import concourse.tile as tile
    from concourse.masks import make_identity

    f32 = mybir.dt.float32
    f32r = mybir.dt.float32r
    Exp = mybir.ActivationFunctionType.Exp
    add = mybir.AluOpType.add
    mult = mybir.AluOpType.mult

    nc = bacc.Bacc("TRN2", target_bir_lowering=False, debug=False,
                   num_devices=NCORE)

    xt_d = nc.dram_tensor("xt", [128, 16 * T], f32r, kind="ExternalInput").ap()
    wt_d = nc.dram_tensor("wt", [128, 16 * 768], f32r, kind="ExternalInput").ap()
    bq_d = nc.dram_tensor("bq", [128, FB], f32, kind="ExternalInput").ap()
    at_d = nc.dram_tensor("at", [128, 20 * 512], f32, kind="ExternalInput").ap()
    cb_d = nc.dram_tensor("cb", [128, 48], f32, kind="ExternalInput").ap()
    pt_d = nc.dram_tensor("pt", [128, 4 * T], f32r, kind="ExternalInput").ap()
    out_d = nc.dram_tensor("out", [T, D], f32, kind="ExternalOutput").ap()

    with tile.TileContext(nc) as tc:
        with tc.tile_pool(name="persist", bufs=1) as pp, \
             tc.tile_pool(name="ps", bufs=3, space="PSUM") as ps_pool, \
             tc.tile_pool(name="po", bufs=2, space="PSUM") as po_pool, \
             tc.tile_pool(name="pv", bufs=2, space="PSUM") as pv_pool:

            qkvT = pp.tile([128, FB * T], f32r, name="qkvT", tag="qkvT")
            bq = pp.tile([128, FB], f32, name="bqs", tag="bqs")
            cb = pp.tile([128, 48], f32, name="cbs", tag="cbs")
            ident = pp.tile([128, 128], f32r, name="ident", tag="ident")
            ones = pp.tile([1, 128], f32r, name="ones1", tag="ones1")
            ones128 = pp.tile([128, 1], f32r, name="ones128", tag="ones128")
            nc.sync.dma_start(bq, bq_d)
            nc.sync.dma_start(cb, cb_d)
            make_identity(nc, ident)
            nc.vector.memset(ones, 1.0)
            nc.vector.memset(ones128, 1.0)

            # ---------------- stage 1: qkvT = W_c @ x^T + b ----------------
            with tc.tile_pool(name="s1w", bufs=1) as s1w, \
                 tc.tile_pool(name="s1x", bufs=2) as s1x:
                wt = s1w.tile([128, 16 * 768], f32r, name="wt", tag="wt")
                for dt_ in range(16):
                    nc.sync.dma_start(wt[:, dt_ * 768:(dt_ + 1) * 768],
                                      wt_d[:, dt_ * 768:(dt_ + 1) * 768])
                for tb in range(4):           # 512-wide t blocks
                    xt = s1x.tile([128, 16 * 512], f32r, name="xt", tag="xt")
                    for dt_ in range(16):
                        nc.sync.dma_start(
                            xt[:, dt_ * 512:(dt_ + 1) * 512],
                            xt_d[:, dt_ * T + tb * 512: dt_ * T + tb * 512 + 512])
                    for fb in range(FB):
                        acc = ps_pool.tile([128, 512], f32, name="acc", tag="ps")
                        for dt_ in range(16):
                            nc.tensor.matmul(
                                acc,
                                wt[:, dt_ * 768 + fb * 128:
                                        dt_ * 768 + fb * 128 + 128],
                                xt[:, dt_ * 512:(dt_ + 1) * 512],
                                start=(dt_ == 0), stop=(dt_ == 15))
                        nc.vector.tensor_scalar_add(
                            qkvT[:, fb * T + tb * 512: fb * T + tb * 512 + 512],
                            acc, bq[:, fb:fb + 1])

            # ---------------- stage 2: attention ----------------
            with tc.tile_pool(name="oTp", bufs=1) as oTp:
                oT = oTp.tile([128, 4 * T], f32r, name="oT", tag="oT")
                with tc.tile_pool(name="s2a", bufs=1) as s2a, \
                     tc.tile_pool(name="s2w", bufs=2) as s2w:
                    at = s2a.tile([128, 20 * 512], f32, name="at", tag="at")
                    for k in range(20):
                        nc.sync.dma_start(at[:, k * 512:(k + 1) * 512],
                                          at_d[:, k * 512:(k + 1) * 512])
                    for h in range(4):
                        qT = qkvT[:, h * T:(h + 1) * T]
                        kT = qkvT[:, 4 * T:5 * T]
                        vT = qkvT[:, 5 * T:6 * T]
                        v = s2w.tile([128, T], f32r, name="v", tag="v")
                        for jt in range(16):
                            pv = pv_pool.tile([128, 128], f32r, name="pv", tag="pv")
                            nc.tensor.transpose(
                                pv, vT[:, jt * 128:(jt + 1) * 128], ident)
                            nc.vector.tensor_copy(
                                v[:, jt * 128:(jt + 1) * 128], pv)
                        for ib in range(4):
                            njb = 4 * (ib + 1)
                            opsum = po_pool.tile([128, 512], f32, name="opsum",
                                                 tag="po")
                            dsum = s2w.tile([128, 512], f32r, name="dsum",
                                            tag="dsum")
                            for jb in range(njb):
                                spsum = ps_pool.tile([128, 512], f32,
                                                     name="spsum", tag="ps")
                                nc.tensor.matmul(
                                    spsum,
                                    kT[:, jb * 128:(jb + 1) * 128],
                                    qT[:, ib * 512:(ib + 1) * 512],
                                    start=True, stop=True)
                                ssb = s2w.tile([128, 512], f32, name="ssb",
                                               tag="ssb", bufs=3)
                                dd = jb - 4 * ib
                                if dd >= 0:   # diagonal band (masked tiles)
                                    nc.vector.tensor_tensor(
                                        ssb, spsum,
                                        at[:, (4 + h * 4 + dd) * 512:
                                              (5 + h * 4 + dd) * 512], add)
                                    bias = 0.0
                                else:         # strictly-lower blocks
                                    nc.vector.tensor_tensor(
                                        ssb, spsum,
                                        at[:, h * 512:(h + 1) * 512], add)
                                    k_ = 4 * ib - jb
                                    bias = cb[:, h * 12 + k_ - 1:
                                                 h * 12 + k_]
                                psb = s2w.tile([128, 512], f32r, name="psb",
                                               tag="psb", bufs=3)
                                nc.scalar.activation(psb, ssb, Exp, bias=bias,
                                                     scale=1.0)
                                if jb == 0:
                                    nc.vector.tensor_copy(dsum, psb)
                                else:
                                    nc.vector.tensor_tensor(dsum, dsum, psb, add)
                                nc.tensor.matmul(
                                    opsum, v[:, jb * 128:(jb + 1) * 128],
                                    psb,
                                    start=(jb == 0), stop=(jb == njb - 1),
                                    skip_group_check=True)
                            # softmax denominator: PE partition reduce 128 -> 1
                            dred = ps_pool.tile([1, 512], f32, name="dred",
                                                tag="ps")
                            nc.tensor.matmul(dred, ones128, dsum,
                                             start=True, stop=True)
                            rsum = s2w.tile([1, 512], f32r, name="rsum",
                                            tag="rsum")
                            with nc.allow_low_precision(
                                    reason="softmax reciprocal in f32r"):
                                nc.vector.reciprocal(rsum, dred)
                            rps = ps_pool.tile([128, 512], f32, name="rps",
                                               tag="ps")
                            nc.tensor.matmul(rps, ones, rsum,
                                             start=True, stop=True)
                            rsb = s2w.tile([128, 512], f32, name="rsb",
                                           tag="rsb")
                            nc.scalar.copy(rsb, rps)
                            nc.vector.tensor_tensor(
                                oT[:, h * T + ib * 512: h * T + ib * 512 + 512],
                                opsum, rsb, mult)

                # ---------------- stage 3: partial proj ----------------
                with tc.tile_pool(name="s3w", bufs=1) as s3w, \
                     tc.tile_pool(name="s3o", bufs=4) as s3o:
                    pt = s3w.tile([128, 4 * T], f32r, name="pt", tag="pt")
                    for k in range(8):
                        nc.sync.dma_start(pt[:, k * 1024:(k + 1) * 1024],
                                          pt_d[:, k * 1024:(k + 1) * 1024])
                    for tb in range(16):
                        for ob in range(4):
                            acc2 = ps_pool.tile([128, 512], f32, name="acc2",
                                                tag="ps")
                            for dt_ in range(4):
                                nc.tensor.matmul(
                                    acc2,
                                    oT[:, dt_ * T + tb * 128:
                                            dt_ * T + tb * 128 + 128],
                                    pt[:, dt_ * T + ob * 512:
                                            dt_ * T + ob * 512 + 512],
                                    start=(dt_ == 0), stop=(dt_ == 3))
                            osb = s3o.tile([128, 512], f32, name="osb",
                                           tag="osb")
                            nc.vector.tensor_copy(osb, acc2)
                            nc.sync.dma_start(
                                out_d[tb * 128:(tb + 1) * 128,
                                      ob * 512:(ob + 1) * 512], osb)

    nc.compile()
    return nc


def get_nc():
    if "nc" not in _CACHE:
        _CACHE["nc"] = _build_nc()
    return _CACHE["nc"]


# --------------------------------------------------------------------------
# host-side packing
# --------------------------------------------------------------------------

def _expected_slopes():
    return 2.0 ** (-8.0 * (np.arange(1, NH + 1) / NH))  # float64


def _check_structure(attn_mask, alibi_bias):
    """Return exact float64 alibi slopes if inputs match the expected
    causal-mask + rank-1 alibi structure, else None."""
    am = np.asarray(attn_mask)
    if am.shape != (1, 1, T, T):
        return None
    if not np.array_equal(am[0, 0], np.tril(np.ones((T, T), dtype=bool))):
        return None
    al = np.asarray(alibi_bias, dtype=np.float32)
    if al.shape != (1, NH, T, T):
        return None
    slopes = _expected_slopes()
    if not np.allclose(al[0, :, 0, 1], slopes.astype(np.float32),
                       rtol=1e-6, atol=1e-8):
        return None
    idx = np.arange(T, dtype=np.float64)
    rel = idx[None, :] - idx[:, None]
    for h in range(NH):
        ref = (slopes[h] * rel).astype(np.float32)
        if not np.array_equal(al[0, h], ref):
            if not np.allclose(al[0, h], ref, rtol=1e-5, atol=1e-4):
                return None
    return slopes


def _pack_core_inputs(x, qkv_w, qkv_b, proj_w, slopes):
    x = np.asarray(x, dtype=np.float32)
    qkv_w = np.asarray(qkv_w, dtype=np.float32)
    qkv_b = np.asarray(qkv_b, dtype=np.float32)
    proj_w = np.asarray(proj_w, dtype=np.float32)
    inv = np.float32(1.0 / math.sqrt(HD))

    xts = []
    for b in range(B):
        xt = np.ascontiguousarray(
            x[b].T.reshape(16, 128, T).transpose(1, 0, 2).reshape(128, 16 * T))
        xts.append(xt)

    per_g = []
    jj = np.arange(128, dtype=np.float64)[:, None]
    ii = np.arange(512, dtype=np.float64)[None, :]
    for g in range(KVH):
        Wq = qkv_w[512 * g:512 * (g + 1)] * inv
        Wk = qkv_w[D + 128 * g: D + 128 * (g + 1)]
        Wv = qkv_w[D + 512 + 128 * g: D + 512 + 128 * (g + 1)]
        Wc = np.concatenate([Wq, Wk, Wv], axis=0)          # [768, 2048]
        wt = np.ascontiguousarray(
            Wc.T.reshape(16, 128, 768).transpose(1, 0, 2).reshape(128, 16 * 768))
        bc = np.concatenate([qkv_b[512 * g:512 * (g + 1)] * inv,
                             qkv_b[D + 128 * g: D + 128 * (g + 1)],
                             qkv_b[D + 512 + 128 * g: D + 512 + 128 * (g + 1)]])
        bqp = np.ascontiguousarray(bc.reshape(FB, 128).T)  # [128, 6]

        at = np.empty((128, 20 * 512), dtype=np.float32)
        cbp = np.empty((128, 48), dtype=np.float32)
        for h in range(GRP):
            s = slopes[4 * g + h]
            at[:, h * 512:(h + 1) * 512] = (s * (jj - ii)).astype(np.float32)
            for dd in range(4):
                A = (s * (jj - ii + 128 * dd)).astype(np.float32)
                A[(jj + 128 * dd - ii) > 0] = np.float32(NEG)
                at[:, (4 + h * 4 + dd) * 512:(5 + h * 4 + dd) * 512] = A
            for k_ in range(1, 13):
                cbp[:, h * 12 + k_ - 1] = np.float32(s * (-128.0 * k_))

        ptp = np.ascontiguousarray(
            proj_w[:, 512 * g:512 * (g + 1)].T
            .reshape(4, 128, T).transpose(1, 0, 2).reshape(128, 4 * T))
        per_g.append({"wt": wt, "bq": bqp, "at": at, "cb": cbp, "pt": ptp})

    in_maps = []
    for c in range(NCORE):
        b, g = divmod(c, KVH)
        m = dict(per_g[g])
        m["xt"] = xts[b]
        in_maps.append(m)
    return in_maps


# --------------------------------------------------------------------------
# numpy fallback (only used if inputs don't match the expected structure)
# --------------------------------------------------------------------------

def _numpy_reference(x, attn_mask, alibi_bias, qkv_w, qkv_b, proj_w, proj_b):
    x = np.asarray(x, dtype=np.float32)
    b, t, c = x.shape
    qkv = x @ qkv_w.T + qkv_b
    q = qkv[..., :D].reshape(b, t, KVH, GRP, HD).transpose(0, 2, 3, 1, 4)
    k = qkv[..., D:D + 512].reshape(b, t, KVH, HD).transpose(0, 2, 1, 3)
    v = qkv[..., D + 512:].reshape(b, t, KVH, HD).transpose(0, 2, 1, 3)
    scale = 1.0 / math.sqrt(HD)
    att = np.einsum("bkgtd,bksd->bkgts", q, k).astype(np.float32) * scale
    att = att + np.asarray(alibi_bias).reshape(1, KVH, GRP, t, t)
    mask = np.asarray(attn_mask)[:, :, None]
    att = np.where(mask, att, -np.inf)
    att = att - att.max(axis=-1, keepdims=True)
    np.exp(att, out=att)
    att /= att.sum(axis=-1, keepdims=True)
    out = np.einsum("bkgts,bksd->bkgtd", att, v)
    out = out.transpose(0, 3, 1, 2, 4).reshape(b, t, c)
    return (out @ proj_w.T + proj_b).astype(np.float32)


# --------------------------------------------------------------------------
# entry point
# --------------------------------------------------------------------------

def kernel(x, attn_mask, alibi_bias, qkv_w, qkv_b, proj_w, proj_b):
    from concourse import bass_utils

    slopes = _check_structure(attn_mask, alibi_bias)
    if slopes is None:
        return _numpy_reference(x, attn_mask, alibi_bias, qkv_w, qkv_b,
                                proj_w, proj_b)

    nc = get_nc()
    in_maps = _pack_core_inputs(x, qkv_w, qkv_b, proj_w, slopes)
    res = bass_utils.run_bass_kernel_spmd(nc, in_maps,
                                          core_ids=list(range(NCORE)))
    proj_b = np.asarray(proj_b, dtype=np.float32)
    out = np.empty((B, T, D), dtype=np.float32)
    for b in range(B):
        acc = res.results[4 * b + 0]["out"].astype(np.float32, copy=True)
        for g in range(1, KVH):
            acc += res.results[4 * b + g]["out"]
        out[b] = acc + proj_b
    return out
